# revision 20
# baseline (speedup 1.0000x reference)
"""GPT2 attention (B=2,S=2048,D=1024,H=16,hd=64, no causal mask) on 8 trn2 cores.

Sharding: core c handles batch b=c//4 and head-group g=c%4 (4 heads).
w_attn columns split per head group (Q pre-scaled by 1/sqrt(hd) on host);
w_proj rows split per head group; host sums the 4 partial c_proj outputs
per batch.

All matmul operands are bf16 (rel-err budget is 2e-2 rms; bf16 lands ~1e-3).
bf16 enables fast weight load, so per-matmul LDWEIGHTS hides behind the
matmul stream, and halves DVE/SBUF/DMA traffic vs f32.

Host-side prep: hid is shipped pre-transposed (hidT [D,S]) so the kernel
needs no PE transposes at all; the c_proj output is produced feature-major
(outT [D,S]) and transposed back on host.

Per-core dataflow:
  A) V seq-major: vps[st,:256] = hidT_tiles.T @ wv  -> vaug [128k, 65] tiles
     (col 64 pre-set to ones -> PV matmul row 64 = softmax denominator)
     Q,K feature-major: qkT[ct][128,2048] = w_slice.T @ hidT (2 heads/tile)
  B) flash loop, per (q-chunk 512, head-pair): 16 k-tiles:
     scores: two row-tiled (K=64) matmuls (head pair runs concurrently in
     the PE array) -> sp [128,1024] PSUM; one ACT exp -> eb bf16;
     PV: op[65, 512+512] += vaug.T @ eb  (row 64 = denominator)
     normalize: denominators batched -> one DVE reciprocal per q-chunk,
     ones-matmul broadcast, DVE multiply -> obar bf16
  C) c_proj feature-major: outT[et,qs] += wp_h.T @ obar_h, drain bf16,
     DMA out. Overlaps stage B of later q-chunks.
"""

import sys

import numpy as np

if "/opt/trn_rl_repo" not in sys.path:
    sys.path.insert(0, "/opt/trn_rl_repo")

S = 2048
D = 1024
P = 128
NH = 4  # heads per core
HD = 64
N_CORES = 8
QC = 512  # q-chunk width
NQC = S // QC  # 4
NKT = S // P  # 16 k-tiles

_CACHE = {}


def _build_program():
    import functools

    import concourse.mybir as mybir
    from concourse import bacc
    from concourse.tile import TileContext

    bf16 = mybir.dt.bfloat16
    f32 = mybir.dt.float32
    AF = mybir.ActivationFunctionType
    ALU = mybir.AluOpType

    nc = bacc.Bacc(None, target_bir_lowering=False, debug=False)
    hidT = nc.declare_dram_parameter("hidT", [D, S], bf16, isOutput=False)
    wqkv = nc.declare_dram_parameter("wqkv", [D, 3 * NH * HD], bf16, isOutput=False)
    wp = nc.declare_dram_parameter("wp", [NH * HD, D], bf16, isOutput=False)
    outT = nc.declare_dram_parameter("outT", [D, S], bf16, isOutput=True)

    with TileContext(nc) as tc:
        with tc.tile_pool(name="const", bufs=1) as constp, \
             tc.tile_pool(name="ebp", bufs=8) as ebp, \
             tc.tile_pool(name="oup", bufs=6) as oup, \
             tc.tile_pool(name="otp", bufs=4) as otp, \
             tc.tile_pool(name="scratch", bufs=2, space="PSUM") as scratch, \
             tc.tile_pool(name="oppsum", bufs=1, space="PSUM") as oppsum:
            ones_bc = constp.tile([P, HD], bf16)
            # vaug: per (h, kt) a [128, 65] block: cols 0..63 = V rows,
            # col 64 = ones (PV denominator row). Pre-set everything to 1;
            # V copies overwrite cols 0..63.
            vaug = constp.tile([P, NH * NKT * 65], bf16)
            qkT = [constp.tile([P, S], bf16, name=f"qkT{i}") for i in range(4)]
            # obar2/wp2: head pairs stacked on partitions (h even: 0:64,
            # h odd: 64:128) so c_proj contracts both heads in one K=128 matmul
            obar2 = [constp.tile([P, S], bf16, name=f"obar2_{i}") for i in range(2)]
            wp2_sb = [constp.tile([P, D], bf16, name=f"wp2_{i}") for i in range(2)]
            # denominators: head h lives on partition 32h so the K=1
            # broadcast matmul's tile_position lands 32-aligned
            dden = constp.tile([97, S], f32)
            drec_f = constp.tile([97, S], f32)
            drec = constp.tile([97, S], bf16)
            hid_sb = [constp.tile([P, S], bf16, name=f"hidT{i}") for i in range(8)]
            w_sb = [
                constp.tile([P, 3 * NH * HD], bf16, name=f"w{i}") for i in range(8)
            ]
            for i in range(8):
                nc.sync.dma_start(out=hid_sb[i][:], in_=hidT[i * P : (i + 1) * P, :])
                nc.gpsimd.dma_start(out=w_sb[i][:], in_=wqkv[i * P : (i + 1) * P, :])
            for hp in range(2):
                for j in range(2):
                    h = 2 * hp + j
                    nc.gpsimd.dma_start(
                        out=wp2_sb[hp][j * HD : (j + 1) * HD, :],
                        in_=wp[h * HD : (h + 1) * HD, :],
                    )
            nc.gpsimd.memset(ones_bc[:], 1.0)
            nc.gpsimd.memset(vaug[:], 1.0)
            nc.gpsimd.memset(dden[:], 1.0)

            # ---- emitters (stage A work routed through the shared scratch
            # pool so it can interleave with the flash loop) ----
            def emit_qk(ct, q):
                # qkT[ct][:, q*QC:...] = w[:, ct-slice].T @ hidT[:, q-slice]
                ps = scratch.tile([P, QC], f32, tag="pp", name="qk_ps")
                for dt_ in range(8):
                    nc.tensor.matmul(
                        ps[:],
                        lhsT=w_sb[dt_][:, ct * P : (ct + 1) * P],
                        rhs=hid_sb[dt_][:, q * QC : (q + 1) * QC],
                        start=(dt_ == 0),
                        stop=(dt_ == 7),
                    )
                nc.vector.tensor_copy(qkT[ct][:, q * QC : (q + 1) * QC], ps[:])

            def emit_vpass(st):
                # V seq-major rows for k-tile st, all 4 heads + ones col
                vps = scratch.tile([P, QC], f32, tag="pp", name="vps")
                for dt_ in range(8):
                    nc.tensor.matmul(
                        vps[:, 0 : NH * HD],
                        lhsT=hid_sb[dt_][:, st * P : (st + 1) * P],
                        rhs=w_sb[dt_][:, 2 * NH * HD : 3 * NH * HD],
                        start=(dt_ == 0),
                        stop=(dt_ == 7),
                    )
                for h in range(NH):
                    base = (h * NKT + st) * 65
                    nc.vector.tensor_copy(
                        vaug[:, base : base + HD], vps[:, h * HD : (h + 1) * HD]
                    )

            def emit_recip(qc):
                q0 = qc * QC
                nc.vector.reciprocal_approx_fast(
                    out=drec_f[:, q0 : q0 + QC], in_=dden[:, q0 : q0 + QC]
                )
                with nc.allow_low_precision(reason="softmax denom bf16"):
                    nc.vector.tensor_copy(
                        drec[:, q0 : q0 + QC], drec_f[:, q0 : q0 + QC]
                    )

            def emit_norm(qc, h):
                q0 = qc * QC
                hp, odd = divmod(h, 2)
                r0 = odd * HD
                rb = scratch.tile([P, QC], f32, tag="pp", name="rb")
                nc.tensor.matmul(
                    rb[r0 : r0 + HD, :],
                    lhsT=ones_bc[32 * h : 32 * h + 1, :],
                    rhs=drec[32 * h : 32 * h + 1, q0 : q0 + QC],
                    start=True, stop=True,
                    tile_position=(32 * h, r0),
                )
                ou = ou_tiles.pop((qc, h))
                with nc.allow_low_precision(reason="softmax normalize bf16"):
                    nc.vector.tensor_tensor(
                        out=obar2[hp][r0 : r0 + HD, q0 : q0 + QC],
                        in0=ou[r0 : r0 + HD, :],
                        in1=rb[r0 : r0 + HD, :],
                        op=ALU.mult,
                    )

            proj_pending = {}

            def emit_proj_a(qc, et):
                q0 = qc * QC
                pp = scratch.tile([P, QC], f32, tag="pp", name="proj_pp")
                proj_pending[(qc, et)] = pp
                nc.tensor.matmul(
                    pp[:],
                    lhsT=wp2_sb[0][:, et * P : (et + 1) * P],
                    rhs=obar2[0][:, q0 : q0 + QC],
                    start=True, stop=False,
                )

            def emit_proj_b(qc, et):
                q0 = qc * QC
                pp = proj_pending.pop((qc, et))
                nc.tensor.matmul(
                    pp[:],
                    lhsT=wp2_sb[1][:, et * P : (et + 1) * P],
                    rhs=obar2[1][:, q0 : q0 + QC],
                    start=False, stop=True,
                )
                ot = otp.tile([P, QC], bf16, tag="ot")
                if et % 2 == 0:
                    nc.vector.tensor_copy(ot[:], pp[:])
                else:
                    nc.scalar.copy(ot[:], pp[:])
                eng = (nc.sync, nc.gpsimd, nc.scalar)[et % 3]
                eng.dma_start(
                    out=outT[et * P : (et + 1) * P, q0 : q0 + QC], in_=ot[:]
                )

            ou_tiles = {}
            filler_q = []

            def flush(n, kt=0):
                if len(filler_q) > 8:
                    n += 1
                for _ in range(min(n, len(filler_q))):
                    filler_q.pop(0)()

            # ---------------- program ----------------
            # Minimal lead: only the QK tiles the first flash block needs up
            # front (ct0/ct2 at q-chunk 0, first two V k-tiles); everything
            # else streams in as deadline-scheduled fillers so the first exp
            # fires ~30us earlier and the PE stays dense (HAM warm).
            emit_qk(0, 0)
            emit_qk(2, 0)
            emit_vpass(0)
            emit_vpass(1)
            vq = list(range(2, NKT))  # pending V k-tiles (deadline: PV kt)
            kq = []                   # pending K-side qk items (deadline: scores kt//4)

            for qc in range(NQC):
                for hp in range(2):
                    q0 = qc * QC
                    if qc == 0 and hp == 0:
                        kq = [(2, 1), (2, 2), (2, 3)]
                    if qc == 0 and hp == 1:
                        emit_qk(1, 0)
                        emit_qk(3, 0)
                        kq = [(3, 1), (3, 2), (3, 3)]
                        # Q-side tiles for later q-chunks: plain fillers
                        for q in (1, 2, 3):
                            filler_q.append(functools.partial(emit_qk, 0, q))
                            filler_q.append(functools.partial(emit_qk, 1, q))
                    h0, h1 = 2 * hp, 2 * hp + 1
                    qT = qkT[hp]
                    kT = qkT[2 + hp]
                    op = oppsum.tile([65, 2 * QC], f32, tag="op")
                    for kt in range(NKT):
                        # K-tiles for scores arrive just ahead of use
                        while kq and kq[0][1] <= (kt + 2) // 4:
                            ct_, q_ = kq.pop(0)
                            emit_qk(ct_, q_)
                        sp = scratch.tile([P, 2 * QC], f32, tag="sp")
                        # row-tiled head pair: h0 in rows 0:64, h1 in 64:128
                        nc.tensor.matmul(
                            sp[:, 0:QC],
                            lhsT=kT[0:HD, kt * P : (kt + 1) * P],
                            rhs=qT[0:HD, q0 : q0 + QC],
                            start=True, stop=True,
                        )
                        nc.tensor.matmul(
                            sp[:, QC : 2 * QC],
                            lhsT=kT[HD:P, kt * P : (kt + 1) * P],
                            rhs=qT[HD:P, q0 : q0 + QC],
                            start=True, stop=True,
                        )
                        eb = ebp.tile([P, 2 * QC], bf16, tag="eb")
                        nc.scalar.activation(eb[:], sp[:], AF.Exp)
                        if qc == 0 and hp == 0:
                            # V k-tiles must be emitted before their PV; keep
                            # a 2-tile lookahead
                            while vq and vq[0] <= min(kt + 2, NKT - 1):
                                emit_vpass(vq.pop(0))
                        else:
                            flush(1, kt)
                        for i, h in enumerate((h0, h1)):
                            base = (h * NKT + kt) * 65
                            nc.tensor.matmul(
                                op[:, i * QC : (i + 1) * QC],
                                lhsT=vaug[:, base : base + 65],
                                rhs=eb[:, i * QC : (i + 1) * QC],
                                start=(kt == 0),
                                stop=(kt == NKT - 1),
                            )
                    # stash denominator row + numerator (bf16)
                    for i, h in enumerate((h0, h1)):
                        nc.vector.tensor_copy(
                            dden[32 * h : 32 * h + 1, q0 : q0 + QC],
                            op[HD : HD + 1, i * QC : (i + 1) * QC],
                        )
                        r0 = (h % 2) * HD
                        ou = oup.tile([P, QC], bf16, tag="ou")
                        nc.vector.tensor_copy(
                            ou[r0 : r0 + HD, :], op[0:HD, i * QC : (i + 1) * QC]
                        )
                        ou_tiles[(qc, h)] = ou
                    # normalize this head pair as soon as its denominators
                    # exist (recip is recomputed after hp1 for rows 64/96)
                    filler_q.append(functools.partial(emit_recip, qc))
                    filler_q.append(functools.partial(emit_norm, qc, h0))
                    filler_q.append(functools.partial(emit_norm, qc, h1))
                    if hp == 1:
                        for et in range(8):
                            filler_q.append(functools.partial(emit_proj_a, qc, et))
                            filler_q.append(functools.partial(emit_proj_b, qc, et))
            flush(len(filler_q))

    nc.compile()
    return nc


def _get_nc():
    if "nc" not in _CACHE:
        _CACHE["nc"] = _build_program()
    return _CACHE["nc"]


def _shard_inputs(hidden_states, w_attn, w_proj):
    import ml_dtypes

    bf16 = ml_dtypes.bfloat16
    scale = 1.0 / np.sqrt(np.float32(HD))
    hidT_b = [
        np.ascontiguousarray(hidden_states[b].T).astype(bf16) for b in range(2)
    ]
    in_maps = []
    for c in range(N_CORES):
        b, g = divmod(c, 4)
        cs = slice(g * NH * HD, (g + 1) * NH * HD)
        wq = w_attn[:, 0:D][:, cs] * scale
        wk = w_attn[:, D : 2 * D][:, cs]
        wv = w_attn[:, 2 * D : 3 * D][:, cs]
        in_maps.append(
            {
                "hidT": hidT_b[b],
                "wqkv": np.ascontiguousarray(
                    np.concatenate([wq, wk, wv], axis=1)
                ).astype(bf16),
                "wp": np.ascontiguousarray(w_proj[cs, :]).astype(bf16),
            }
        )
    return in_maps


def run(hidden_states, w_attn, w_proj, trace=False):
    from concourse.bass_utils import run_bass_kernel_spmd

    nc = _get_nc()
    in_maps = _shard_inputs(hidden_states, w_attn, w_proj)
    res = run_bass_kernel_spmd(nc, in_maps, list(range(N_CORES)), trace=trace)
    parts = [res.results[c]["outT"].astype(np.float32).T for c in range(N_CORES)]
    out = np.stack(
        [
            parts[0] + parts[1] + parts[2] + parts[3],
            parts[4] + parts[5] + parts[6] + parts[7],
        ]
    ).astype(np.float32)
    return out, res


def kernel(hidden_states, w_attn, w_proj):
    out, _ = run(
        np.asarray(hidden_states), np.asarray(w_attn), np.asarray(w_proj)
    )
    return out


# revision 21
# speedup vs baseline: 1.0428x; 1.0428x over previous
"""GPT2 attention (B=2,S=2048,D=1024,H=16,hd=64, no causal mask) on 8 trn2 cores.

Sharding: core c handles batch b=c//4 and head-group g=c%4 (4 heads).
w_attn columns split per head group (Q pre-scaled by 1/sqrt(hd) on host);
w_proj rows split per head group; host sums the 4 partial c_proj outputs
per batch.

All matmul operands are bf16 (rel-err budget is 2e-2 rms; bf16 lands ~1e-3).
bf16 enables fast weight load, so per-matmul LDWEIGHTS hides behind the
matmul stream, and halves DVE/SBUF/DMA traffic vs f32.

Host-side prep: hid is shipped pre-transposed (hidT [D,S]) so the kernel
needs no PE transposes at all; the c_proj output is produced feature-major
(outT [D,S]) and transposed back on host.

Per-core dataflow:
  A) V seq-major: vps[st,:256] = hidT_tiles.T @ wv  -> vaug [128k, 65] tiles
     (col 64 pre-set to ones -> PV matmul row 64 = softmax denominator)
     Q,K feature-major: qkT[ct][128,2048] = w_slice.T @ hidT (2 heads/tile)
  B) flash loop, per (q-chunk 512, head-pair): 16 k-tiles:
     scores: two row-tiled (K=64) matmuls (head pair runs concurrently in
     the PE array) -> sp [128,1024] PSUM; one ACT exp -> eb bf16;
     PV: op[65, 512+512] += vaug.T @ eb  (row 64 = denominator)
     normalize: denominators batched -> one DVE reciprocal per q-chunk,
     ones-matmul broadcast, DVE multiply -> obar bf16
  C) c_proj feature-major: outT[et,qs] += wp_h.T @ obar_h, drain bf16,
     DMA out. Overlaps stage B of later q-chunks.
"""

import sys

import numpy as np

if "/opt/trn_rl_repo" not in sys.path:
    sys.path.insert(0, "/opt/trn_rl_repo")

S = 2048
D = 1024
P = 128
NH = 4  # heads per core
HD = 64
N_CORES = 8
QC = 512  # q-chunk width
NQC = S // QC  # 4
NKT = S // P  # 16 k-tiles

_CACHE = {}


def _build_program():
    import functools

    import concourse.mybir as mybir
    from concourse import bacc
    from concourse.tile import TileContext

    bf16 = mybir.dt.bfloat16
    f32 = mybir.dt.float32
    AF = mybir.ActivationFunctionType
    ALU = mybir.AluOpType

    nc = bacc.Bacc(None, target_bir_lowering=False, debug=False)
    hidT = nc.declare_dram_parameter("hidT", [D, S], bf16, isOutput=False)
    wqkv = nc.declare_dram_parameter("wqkv", [D, 3 * NH * HD], bf16, isOutput=False)
    wp = nc.declare_dram_parameter("wp", [NH * HD, D], bf16, isOutput=False)
    outT = nc.declare_dram_parameter("outT", [D, S], bf16, isOutput=True)

    with TileContext(nc) as tc:
        with tc.tile_pool(name="const", bufs=1) as constp, \
             tc.tile_pool(name="ebp", bufs=8) as ebp, \
             tc.tile_pool(name="oup", bufs=6) as oup, \
             tc.tile_pool(name="otp", bufs=4) as otp, \
             tc.tile_pool(name="accp", bufs=8) as accp, \
             tc.tile_pool(name="scratch", bufs=2, space="PSUM") as scratch, \
             tc.tile_pool(name="oppsum", bufs=1, space="PSUM") as oppsum:
            ones_bc = constp.tile([P, HD], bf16)
            # vaug: per (h, kt) a [128, 65] block: cols 0..63 = V rows,
            # col 64 = ones (PV denominator row). Pre-set everything to 1;
            # V copies overwrite cols 0..63.
            vaug = constp.tile([P, NH * NKT * 65], bf16)
            qkT = [constp.tile([P, S], bf16, name=f"qkT{i}") for i in range(4)]
            # obar2/wp2: head pairs stacked on partitions (h even: 0:64,
            # h odd: 64:128) so c_proj contracts both heads in one K=128 matmul
            obar2 = [constp.tile([P, S], bf16, name=f"obar2_{i}") for i in range(2)]
            wp2_sb = [constp.tile([P, D], bf16, name=f"wp2_{i}") for i in range(2)]
            # denominators: head h lives on partition 32h so the K=1
            # broadcast matmul's tile_position lands 32-aligned
            dden = constp.tile([97, S], f32)
            drec_f = constp.tile([97, S], f32)
            drec = constp.tile([97, S], bf16)
            hid_sb = [constp.tile([P, S], bf16, name=f"hidT{i}") for i in range(8)]
            w_sb = [
                constp.tile([P, 3 * NH * HD], bf16, name=f"w{i}") for i in range(8)
            ]
            for i in range(8):
                nc.sync.dma_start(
                    out=hid_sb[i][:, 0 : S // 2],
                    in_=hidT[i * P : (i + 1) * P, 0 : S // 2],
                )
                nc.gpsimd.dma_start(out=w_sb[i][:], in_=wqkv[i * P : (i + 1) * P, :])
            for i in range(8):
                nc.sync.dma_start(
                    out=hid_sb[i][:, S // 2 : S],
                    in_=hidT[i * P : (i + 1) * P, S // 2 : S],
                )
            for hp in range(2):
                for j in range(2):
                    h = 2 * hp + j
                    nc.gpsimd.dma_start(
                        out=wp2_sb[hp][j * HD : (j + 1) * HD, :],
                        in_=wp[h * HD : (h + 1) * HD, :],
                    )
            nc.gpsimd.memset(ones_bc[:], 1.0)
            nc.gpsimd.memset(vaug[:], 1.0)
            nc.gpsimd.memset(dden[:], 1.0)

            # ---- emitters (stage A work routed through the shared scratch
            # pool so it can interleave with the flash loop) ----
            def emit_qk(ct, q):
                # qkT[ct][:, q*QC:...] = w[:, ct-slice].T @ hidT[:, q-slice]
                ps = scratch.tile([P, QC], f32, tag="pp", name="qk_ps")
                for dt_ in range(8):
                    nc.tensor.matmul(
                        ps[:],
                        lhsT=w_sb[dt_][:, ct * P : (ct + 1) * P],
                        rhs=hid_sb[dt_][:, q * QC : (q + 1) * QC],
                        start=(dt_ == 0),
                        stop=(dt_ == 7),
                    )
                nc.vector.tensor_copy(qkT[ct][:, q * QC : (q + 1) * QC], ps[:])

            def emit_vpass(st):
                # V seq-major rows for k-tile st, all 4 heads + ones col
                vps = scratch.tile([P, QC], f32, tag="pp", name="vps")
                for dt_ in range(8):
                    nc.tensor.matmul(
                        vps[:, 0 : NH * HD],
                        lhsT=hid_sb[dt_][:, st * P : (st + 1) * P],
                        rhs=w_sb[dt_][:, 2 * NH * HD : 3 * NH * HD],
                        start=(dt_ == 0),
                        stop=(dt_ == 7),
                    )
                for h in range(NH):
                    base = (h * NKT + st) * 65
                    nc.vector.tensor_copy(
                        vaug[:, base : base + HD], vps[:, h * HD : (h + 1) * HD]
                    )

            def emit_recip(qc):
                q0 = qc * QC
                nc.vector.reciprocal_approx_fast(
                    out=drec_f[:, q0 : q0 + QC], in_=dden[:, q0 : q0 + QC]
                )
                with nc.allow_low_precision(reason="softmax denom bf16"):
                    nc.vector.tensor_copy(
                        drec[:, q0 : q0 + QC], drec_f[:, q0 : q0 + QC]
                    )

            def emit_norm(qc, h):
                q0 = qc * QC
                hp, odd = divmod(h, 2)
                r0 = odd * HD
                rb = scratch.tile([P, QC], f32, tag="pp", name="rb")
                nc.tensor.matmul(
                    rb[r0 : r0 + HD, :],
                    lhsT=ones_bc[32 * h : 32 * h + 1, :],
                    rhs=drec[32 * h : 32 * h + 1, q0 : q0 + QC],
                    start=True, stop=True,
                    tile_position=(32 * h, r0),
                )
                ou = ou_tiles.pop((qc, h))
                with nc.allow_low_precision(reason="softmax normalize bf16"):
                    nc.vector.tensor_tensor(
                        out=obar2[hp][r0 : r0 + HD, q0 : q0 + QC],
                        in0=ou[r0 : r0 + HD, :],
                        in1=rb[r0 : r0 + HD, :],
                        op=ALU.mult,
                    )

            proj_pending = {}

            def emit_proj_a(qc, et):
                q0 = qc * QC
                pp = scratch.tile([P, QC], f32, tag="pp", name="proj_pp")
                proj_pending[(qc, et)] = pp
                nc.tensor.matmul(
                    pp[:],
                    lhsT=wp2_sb[0][:, et * P : (et + 1) * P],
                    rhs=obar2[0][:, q0 : q0 + QC],
                    start=True, stop=False,
                )

            def emit_proj_b(qc, et):
                q0 = qc * QC
                pp = proj_pending.pop((qc, et))
                nc.tensor.matmul(
                    pp[:],
                    lhsT=wp2_sb[1][:, et * P : (et + 1) * P],
                    rhs=obar2[1][:, q0 : q0 + QC],
                    start=False, stop=True,
                )
                ot = otp.tile([P, QC], bf16, tag="ot")
                nc.vector.tensor_copy(ot[:], pp[:])
                nc.sync.dma_start(
                    out=outT[et * P : (et + 1) * P, q0 : q0 + QC], in_=ot[:]
                )

            acc_tiles = {}

            def emit_proj3_a(et):
                q0 = 3 * QC
                pp = scratch.tile([P, QC], f32, tag="pp", name="p3a")
                nc.tensor.matmul(
                    pp[:],
                    lhsT=wp2_sb[0][:, et * P : (et + 1) * P],
                    rhs=obar2[0][:, q0 : q0 + QC],
                    start=True, stop=True,
                )
                acc = accp.tile([P, QC], f32, tag="acc")
                nc.vector.tensor_copy(acc[:], pp[:])
                acc_tiles[et] = acc

            def emit_proj3_b(et):
                q0 = 3 * QC
                pp = scratch.tile([P, QC], f32, tag="pp", name="p3b")
                nc.tensor.matmul(
                    pp[:],
                    lhsT=wp2_sb[1][:, et * P : (et + 1) * P],
                    rhs=obar2[1][:, q0 : q0 + QC],
                    start=True, stop=True,
                )
                ot = otp.tile([P, QC], bf16, tag="ot")
                acc = acc_tiles.pop(et)
                with nc.allow_low_precision(reason="c_proj half-sum bf16"):
                    nc.vector.tensor_tensor(
                        out=ot[:], in0=acc[:], in1=pp[:], op=ALU.add
                    )
                nc.sync.dma_start(
                    out=outT[et * P : (et + 1) * P, q0 : q0 + QC], in_=ot[:]
                )

            ou_tiles = {}
            filler_q = []

            def flush(n, kt=0):
                if len(filler_q) > 8:
                    n += 1
                for _ in range(min(n, len(filler_q))):
                    filler_q.pop(0)()

            # ---------------- program ----------------
            # Minimal lead: only the QK tiles the first flash block needs up
            # front (ct0/ct2 at q-chunk 0, first two V k-tiles); everything
            # else streams in as deadline-scheduled fillers so the first exp
            # fires ~30us earlier and the PE stays dense (HAM warm).
            emit_qk(0, 0)
            emit_qk(2, 0)
            emit_vpass(0)
            emit_vpass(1)
            vq = list(range(2, NKT))  # pending V k-tiles (deadline: PV kt)
            kq = []                   # pending K-side qk items (deadline: scores kt//4)

            for qc in range(NQC):
                for hp in range(2):
                    q0 = qc * QC
                    if qc == 0 and hp == 0:
                        kq = [(2, 1), (2, 2), (2, 3)]
                    if qc == 0 and hp == 1:
                        emit_qk(1, 0)
                        emit_qk(3, 0)
                        kq = [(3, 1), (3, 2), (3, 3)]
                        # Q-side tiles for later q-chunks: plain fillers
                        for q in (1, 2, 3):
                            filler_q.append(functools.partial(emit_qk, 0, q))
                            filler_q.append(functools.partial(emit_qk, 1, q))
                    h0, h1 = 2 * hp, 2 * hp + 1
                    qT = qkT[hp]
                    kT = qkT[2 + hp]
                    op = oppsum.tile([65, 2 * QC], f32, tag="op")
                    for kt in range(NKT):
                        # K-tiles for scores arrive just ahead of use
                        while kq and kq[0][1] <= (kt + 2) // 4:
                            ct_, q_ = kq.pop(0)
                            emit_qk(ct_, q_)
                        sp = scratch.tile([P, 2 * QC], f32, tag="sp")
                        # row-tiled head pair: h0 in rows 0:64, h1 in 64:128
                        nc.tensor.matmul(
                            sp[:, 0:QC],
                            lhsT=kT[0:HD, kt * P : (kt + 1) * P],
                            rhs=qT[0:HD, q0 : q0 + QC],
                            start=True, stop=True,
                        )
                        nc.tensor.matmul(
                            sp[:, QC : 2 * QC],
                            lhsT=kT[HD:P, kt * P : (kt + 1) * P],
                            rhs=qT[HD:P, q0 : q0 + QC],
                            start=True, stop=True,
                        )
                        eb = ebp.tile([P, 2 * QC], bf16, tag="eb")
                        nc.scalar.activation(eb[:], sp[:], AF.Exp)
                        if qc == 0 and hp == 0:
                            # V k-tiles must be emitted before their PV; keep
                            # a 2-tile lookahead
                            while vq and vq[0] <= min(kt + 2, NKT - 1):
                                emit_vpass(vq.pop(0))
                        else:
                            flush(1, kt)
                        for i, h in enumerate((h0, h1)):
                            base = (h * NKT + kt) * 65
                            nc.tensor.matmul(
                                op[:, i * QC : (i + 1) * QC],
                                lhsT=vaug[:, base : base + 65],
                                rhs=eb[:, i * QC : (i + 1) * QC],
                                start=(kt == 0),
                                stop=(kt == NKT - 1),
                            )
                    # stash denominator row + numerator (bf16)
                    for i, h in enumerate((h0, h1)):
                        nc.vector.tensor_copy(
                            dden[32 * h : 32 * h + 1, q0 : q0 + QC],
                            op[HD : HD + 1, i * QC : (i + 1) * QC],
                        )
                        r0 = (h % 2) * HD
                        ou = oup.tile([P, QC], bf16, tag="ou")
                        nc.vector.tensor_copy(
                            ou[r0 : r0 + HD, :], op[0:HD, i * QC : (i + 1) * QC]
                        )
                        ou_tiles[(qc, h)] = ou
                    # normalize this head pair as soon as its denominators
                    # exist (recip is recomputed after hp1 for rows 64/96)
                    filler_q.append(functools.partial(emit_recip, qc))
                    filler_q.append(functools.partial(emit_norm, qc, h0))
                    filler_q.append(functools.partial(emit_norm, qc, h1))
                    if qc == NQC - 1 and hp == 0:
                        # last q-chunk: stage the hp0 half of c_proj during
                        # the final block so the tail is one matmul + add
                        for et in range(8):
                            filler_q.append(functools.partial(emit_proj3_a, et))
                    elif hp == 1 and qc < NQC - 1:
                        for et in range(8):
                            filler_q.append(functools.partial(emit_proj_a, qc, et))
                            filler_q.append(functools.partial(emit_proj_b, qc, et))
                    elif hp == 1:
                        for et in range(8):
                            filler_q.append(functools.partial(emit_proj3_b, et))
            flush(len(filler_q))

    nc.compile()
    return nc


def _get_nc():
    if "nc" not in _CACHE:
        _CACHE["nc"] = _build_program()
    return _CACHE["nc"]


def _shard_inputs(hidden_states, w_attn, w_proj):
    import ml_dtypes

    bf16 = ml_dtypes.bfloat16
    scale = 1.0 / np.sqrt(np.float32(HD))
    hidT_b = [
        np.ascontiguousarray(hidden_states[b].T).astype(bf16) for b in range(2)
    ]
    in_maps = []
    for c in range(N_CORES):
        b, g = divmod(c, 4)
        cs = slice(g * NH * HD, (g + 1) * NH * HD)
        wq = w_attn[:, 0:D][:, cs] * scale
        wk = w_attn[:, D : 2 * D][:, cs]
        wv = w_attn[:, 2 * D : 3 * D][:, cs]
        in_maps.append(
            {
                "hidT": hidT_b[b],
                "wqkv": np.ascontiguousarray(
                    np.concatenate([wq, wk, wv], axis=1)
                ).astype(bf16),
                "wp": np.ascontiguousarray(w_proj[cs, :]).astype(bf16),
            }
        )
    return in_maps


def run(hidden_states, w_attn, w_proj, trace=False):
    from concourse.bass_utils import run_bass_kernel_spmd

    nc = _get_nc()
    in_maps = _shard_inputs(hidden_states, w_attn, w_proj)
    res = run_bass_kernel_spmd(nc, in_maps, list(range(N_CORES)), trace=trace)
    parts = [res.results[c]["outT"].astype(np.float32).T for c in range(N_CORES)]
    out = np.stack(
        [
            parts[0] + parts[1] + parts[2] + parts[3],
            parts[4] + parts[5] + parts[6] + parts[7],
        ]
    ).astype(np.float32)
    return out, res


def kernel(hidden_states, w_attn, w_proj):
    out, _ = run(
        np.asarray(hidden_states), np.asarray(w_attn), np.asarray(w_proj)
    )
    return out


# revision 22
# speedup vs baseline: 1.0656x; 1.0218x over previous
"""GPT2 attention (B=2,S=2048,D=1024,H=16,hd=64, no causal mask) on 8 trn2 cores.

Sharding: core c handles batch b=c//4 and head-group g=c%4 (4 heads).
w_attn columns split per head group (Q pre-scaled by 1/sqrt(hd) on host);
w_proj rows split per head group; host sums the 4 partial c_proj outputs
per batch.

All matmul operands are bf16 (rel-err budget is 2e-2 rms; bf16 lands ~1e-3).
bf16 enables fast weight load, so per-matmul LDWEIGHTS hides behind the
matmul stream, and halves DVE/SBUF/DMA traffic vs f32.

Host-side prep: hid is shipped pre-transposed (hidT [D,S]) so the kernel
needs no PE transposes at all; the c_proj output is produced feature-major
(outT [D,S]) and transposed back on host.

Per-core dataflow:
  A) V seq-major: vps[st,:256] = hidT_tiles.T @ wv  -> vaug [128k, 65] tiles
     (col 64 pre-set to ones -> PV matmul row 64 = softmax denominator)
     Q,K feature-major: qkT[ct][128,2048] = w_slice.T @ hidT (2 heads/tile)
  B) flash loop, per (q-chunk 512, head-pair): 16 k-tiles:
     scores: two row-tiled (K=64) matmuls (head pair runs concurrently in
     the PE array) -> sp [128,1024] PSUM; one ACT exp -> eb bf16;
     PV: op[65, 512+512] += vaug.T @ eb  (row 64 = denominator)
     normalize: denominators batched -> one DVE reciprocal per q-chunk,
     ones-matmul broadcast, DVE multiply -> obar bf16
  C) c_proj feature-major: outT[et,qs] += wp_h.T @ obar_h, drain bf16,
     DMA out. Overlaps stage B of later q-chunks.
"""

import sys

import numpy as np

if "/opt/trn_rl_repo" not in sys.path:
    sys.path.insert(0, "/opt/trn_rl_repo")

S = 2048
D = 1024
P = 128
NH = 4  # heads per core
HD = 64
N_CORES = 8
QC = 512  # q-chunk width
NQC = S // QC  # 4
NKT = S // P  # 16 k-tiles

_CACHE = {}


def _build_program():
    import functools

    import concourse.mybir as mybir
    from concourse import bacc
    from concourse.tile import TileContext

    bf16 = mybir.dt.bfloat16
    f32 = mybir.dt.float32
    AF = mybir.ActivationFunctionType
    ALU = mybir.AluOpType

    nc = bacc.Bacc(None, target_bir_lowering=False, debug=False)
    hidT = nc.declare_dram_parameter("hidT", [D, S], bf16, isOutput=False)
    wqkv = nc.declare_dram_parameter("wqkv", [D, 3 * NH * HD], bf16, isOutput=False)
    wp = nc.declare_dram_parameter("wp", [NH * HD, D], bf16, isOutput=False)
    outT = nc.declare_dram_parameter("outT", [D, S], bf16, isOutput=True)

    with TileContext(nc) as tc:
        with tc.tile_pool(name="const", bufs=1) as constp, \
             tc.tile_pool(name="ebp", bufs=8) as ebp, \
             tc.tile_pool(name="oup", bufs=6) as oup, \
             tc.tile_pool(name="otp", bufs=4) as otp, \
             tc.tile_pool(name="accp", bufs=8) as accp, \
             tc.tile_pool(name="scratch", bufs=2, space="PSUM") as scratch, \
             tc.tile_pool(name="oppsum", bufs=1, space="PSUM") as oppsum:
            ones_bc = constp.tile([P, HD], bf16)
            # vaug: per (h, kt) a [128, 65] block: cols 0..63 = V rows,
            # col 64 = ones (PV denominator row). Pre-set everything to 1;
            # V copies overwrite cols 0..63.
            vaug = constp.tile([P, NH * NKT * 65], bf16)
            qkT = [constp.tile([P, S], bf16, name=f"qkT{i}") for i in range(4)]
            # obar2/wp2: head pairs stacked on partitions (h even: 0:64,
            # h odd: 64:128) so c_proj contracts both heads in one K=128 matmul
            obar2 = [constp.tile([P, S], bf16, name=f"obar2_{i}") for i in range(2)]
            wp2_sb = [constp.tile([P, D], bf16, name=f"wp2_{i}") for i in range(2)]
            # denominators: head h lives on partition 32h so the K=1
            # broadcast matmul's tile_position lands 32-aligned
            dden = constp.tile([97, S], f32)
            drec_f = constp.tile([97, S], f32)
            drec = constp.tile([97, S], bf16)
            hid_sb = [constp.tile([P, S], bf16, name=f"hidT{i}") for i in range(8)]
            w_sb = [
                constp.tile([P, 3 * NH * HD], bf16, name=f"w{i}") for i in range(8)
            ]
            for i in range(8):
                nc.sync.dma_start(
                    out=hid_sb[i][:, 0 : S // 2],
                    in_=hidT[i * P : (i + 1) * P, 0 : S // 2],
                )
                nc.gpsimd.dma_start(out=w_sb[i][:], in_=wqkv[i * P : (i + 1) * P, :])
            for i in range(8):
                nc.sync.dma_start(
                    out=hid_sb[i][:, S // 2 : S],
                    in_=hidT[i * P : (i + 1) * P, S // 2 : S],
                )
            for hp in range(2):
                for j in range(2):
                    h = 2 * hp + j
                    nc.gpsimd.dma_start(
                        out=wp2_sb[hp][j * HD : (j + 1) * HD, :],
                        in_=wp[h * HD : (h + 1) * HD, :],
                    )
            nc.gpsimd.memset(ones_bc[:], 1.0)
            nc.gpsimd.memset(vaug[:], 1.0)
            nc.gpsimd.memset(dden[:], 1.0)

            # ---- emitters (stage A work routed through the shared scratch
            # pool so it can interleave with the flash loop) ----
            def emit_qk(ct, q):
                # qkT[ct][:, q*QC:...] = w[:, ct-slice].T @ hidT[:, q-slice]
                ps = scratch.tile([P, QC], f32, tag="pp", name="qk_ps")
                for dt_ in range(8):
                    nc.tensor.matmul(
                        ps[:],
                        lhsT=w_sb[dt_][:, ct * P : (ct + 1) * P],
                        rhs=hid_sb[dt_][:, q * QC : (q + 1) * QC],
                        start=(dt_ == 0),
                        stop=(dt_ == 7),
                    )
                nc.vector.tensor_copy(qkT[ct][:, q * QC : (q + 1) * QC], ps[:])

            def emit_vpass(st):
                # V seq-major rows for k-tile st, all 4 heads + ones col
                vps = scratch.tile([P, QC], f32, tag="pp", name="vps")
                for dt_ in range(8):
                    nc.tensor.matmul(
                        vps[:, 0 : NH * HD],
                        lhsT=hid_sb[dt_][:, st * P : (st + 1) * P],
                        rhs=w_sb[dt_][:, 2 * NH * HD : 3 * NH * HD],
                        start=(dt_ == 0),
                        stop=(dt_ == 7),
                    )
                for h in range(NH):
                    base = (h * NKT + st) * 65
                    nc.vector.tensor_copy(
                        vaug[:, base : base + HD], vps[:, h * HD : (h + 1) * HD]
                    )

            def emit_recip(qc):
                q0 = qc * QC
                nc.vector.reciprocal_approx_fast(
                    out=drec_f[:, q0 : q0 + QC], in_=dden[:, q0 : q0 + QC]
                )
                with nc.allow_low_precision(reason="softmax denom bf16"):
                    nc.vector.tensor_copy(
                        drec[:, q0 : q0 + QC], drec_f[:, q0 : q0 + QC]
                    )

            def emit_norm(qc, h):
                q0 = qc * QC
                hp, odd = divmod(h, 2)
                r0 = odd * HD
                rb = scratch.tile([P, QC], f32, tag="pp", name="rb")
                nc.tensor.matmul(
                    rb[r0 : r0 + HD, :],
                    lhsT=ones_bc[32 * h : 32 * h + 1, :],
                    rhs=drec[32 * h : 32 * h + 1, q0 : q0 + QC],
                    start=True, stop=True,
                    tile_position=(32 * h, r0),
                )
                ou = ou_tiles.pop((qc, h))
                with nc.allow_low_precision(reason="softmax normalize bf16"):
                    nc.vector.tensor_tensor(
                        out=obar2[hp][r0 : r0 + HD, q0 : q0 + QC],
                        in0=ou[r0 : r0 + HD, :],
                        in1=rb[r0 : r0 + HD, :],
                        op=ALU.mult,
                    )

            proj_pending = {}

            def emit_proj_a(qc, et):
                q0 = qc * QC
                pp = scratch.tile([P, QC], f32, tag="pp", name="proj_pp")
                proj_pending[(qc, et)] = pp
                nc.tensor.matmul(
                    pp[:],
                    lhsT=wp2_sb[0][:, et * P : (et + 1) * P],
                    rhs=obar2[0][:, q0 : q0 + QC],
                    start=True, stop=False,
                )

            def emit_proj_b(qc, et):
                q0 = qc * QC
                pp = proj_pending.pop((qc, et))
                nc.tensor.matmul(
                    pp[:],
                    lhsT=wp2_sb[1][:, et * P : (et + 1) * P],
                    rhs=obar2[1][:, q0 : q0 + QC],
                    start=False, stop=True,
                )
                ot = otp.tile([P, QC], bf16, tag="ot")
                nc.vector.tensor_copy(ot[:], pp[:])
                (nc.sync if et % 2 == 0 else nc.gpsimd).dma_start(
                    out=outT[et * P : (et + 1) * P, q0 : q0 + QC], in_=ot[:]
                )

            acc_tiles = {}

            def emit_proj3_a(et):
                q0 = 3 * QC
                pp = scratch.tile([P, QC], f32, tag="pp", name="p3a")
                nc.tensor.matmul(
                    pp[:],
                    lhsT=wp2_sb[0][:, et * P : (et + 1) * P],
                    rhs=obar2[0][:, q0 : q0 + QC],
                    start=True, stop=True,
                )
                acc = accp.tile([P, QC], f32, tag="acc")
                nc.vector.tensor_copy(acc[:], pp[:])
                acc_tiles[et] = acc

            def emit_proj3_b(et):
                q0 = 3 * QC
                pp = scratch.tile([P, QC], f32, tag="pp", name="p3b")
                nc.tensor.matmul(
                    pp[:],
                    lhsT=wp2_sb[1][:, et * P : (et + 1) * P],
                    rhs=obar2[1][:, q0 : q0 + QC],
                    start=True, stop=True,
                )
                ot = otp.tile([P, QC], bf16, tag="ot")
                acc = acc_tiles.pop(et)
                with nc.allow_low_precision(reason="c_proj half-sum bf16"):
                    nc.vector.tensor_tensor(
                        out=ot[:], in0=acc[:], in1=pp[:], op=ALU.add
                    )
                (nc.sync if et % 2 == 0 else nc.gpsimd).dma_start(
                    out=outT[et * P : (et + 1) * P, q0 : q0 + QC], in_=ot[:]
                )

            ou_tiles = {}
            filler_q = []

            def flush(n, kt=0):
                if len(filler_q) > 12:
                    n += 1
                for _ in range(min(n, len(filler_q))):
                    filler_q.pop(0)()

            # ---------------- program ----------------
            # Minimal lead: only the QK tiles the first flash block needs up
            # front (ct0/ct2 at q-chunk 0, first two V k-tiles); everything
            # else streams in as deadline-scheduled fillers so the first exp
            # fires ~30us earlier and the PE stays dense (HAM warm).
            ps00 = scratch.tile([P, QC], f32, tag="pp", name="ps00")
            ps20 = scratch.tile([P, QC], f32, tag="pp", name="ps20")
            for dt_ in range(8):
                for ct, ps in ((0, ps00), (2, ps20)):
                    nc.tensor.matmul(
                        ps[:],
                        lhsT=w_sb[dt_][:, ct * P : (ct + 1) * P],
                        rhs=hid_sb[dt_][:, 0:QC],
                        start=(dt_ == 0),
                        stop=(dt_ == 7),
                    )
            nc.vector.tensor_copy(qkT[0][:, 0:QC], ps00[:])
            nc.vector.tensor_copy(qkT[2][:, 0:QC], ps20[:])
            emit_vpass(0)
            emit_vpass(1)
            vq = list(range(2, NKT))  # pending V k-tiles (deadline: PV kt)
            kq = []                   # pending K-side qk items (deadline: scores kt//4)

            for qc in range(NQC):
                for hp in range(2):
                    q0 = qc * QC
                    if qc == 0 and hp == 0:
                        kq = [(2, 1), (2, 2), (2, 3)]
                    if qc == 0 and hp == 1:
                        emit_qk(1, 0)
                        emit_qk(3, 0)
                        kq = [(3, 1), (3, 2), (3, 3)]
                        # Q-side tiles for later q-chunks: plain fillers
                        for q in (1, 2, 3):
                            filler_q.append(functools.partial(emit_qk, 0, q))
                            filler_q.append(functools.partial(emit_qk, 1, q))
                    h0, h1 = 2 * hp, 2 * hp + 1
                    qT = qkT[hp]
                    kT = qkT[2 + hp]
                    op = oppsum.tile([65, 2 * QC], f32, tag="op")
                    for kt in range(NKT):
                        # K-tiles for scores arrive just ahead of use
                        while kq and kq[0][1] <= (kt + 2) // 4:
                            ct_, q_ = kq.pop(0)
                            emit_qk(ct_, q_)
                        sp = scratch.tile([P, 2 * QC], f32, tag="sp")
                        # row-tiled head pair: h0 in rows 0:64, h1 in 64:128
                        nc.tensor.matmul(
                            sp[:, 0:QC],
                            lhsT=kT[0:HD, kt * P : (kt + 1) * P],
                            rhs=qT[0:HD, q0 : q0 + QC],
                            start=True, stop=True,
                        )
                        nc.tensor.matmul(
                            sp[:, QC : 2 * QC],
                            lhsT=kT[HD:P, kt * P : (kt + 1) * P],
                            rhs=qT[HD:P, q0 : q0 + QC],
                            start=True, stop=True,
                        )
                        eb = ebp.tile([P, 2 * QC], bf16, tag="eb")
                        nc.scalar.activation(eb[:], sp[:], AF.Exp)
                        if qc == 0 and hp == 0:
                            # V k-tiles must be emitted before their PV; keep
                            # a 2-tile lookahead
                            while vq and vq[0] <= min(kt + 2, NKT - 1):
                                emit_vpass(vq.pop(0))
                        else:
                            flush(1, kt)
                        for i, h in enumerate((h0, h1)):
                            base = (h * NKT + kt) * 65
                            nc.tensor.matmul(
                                op[:, i * QC : (i + 1) * QC],
                                lhsT=vaug[:, base : base + 65],
                                rhs=eb[:, i * QC : (i + 1) * QC],
                                start=(kt == 0),
                                stop=(kt == NKT - 1),
                            )
                    # stash denominator row + numerator (bf16)
                    for i, h in enumerate((h0, h1)):
                        nc.vector.tensor_copy(
                            dden[32 * h : 32 * h + 1, q0 : q0 + QC],
                            op[HD : HD + 1, i * QC : (i + 1) * QC],
                        )
                        r0 = (h % 2) * HD
                        ou = oup.tile([P, QC], bf16, tag="ou")
                        nc.vector.tensor_copy(
                            ou[r0 : r0 + HD, :], op[0:HD, i * QC : (i + 1) * QC]
                        )
                        ou_tiles[(qc, h)] = ou
                    # normalize this head pair as soon as its denominators
                    # exist (recip is recomputed after hp1 for rows 64/96)
                    filler_q.append(functools.partial(emit_recip, qc))
                    filler_q.append(functools.partial(emit_norm, qc, h0))
                    filler_q.append(functools.partial(emit_norm, qc, h1))
                    if qc == NQC - 1 and hp == 0:
                        # last q-chunk: stage the hp0 half of c_proj during
                        # the final block so the tail is one matmul + add
                        for et in range(8):
                            filler_q.append(functools.partial(emit_proj3_a, et))
                    elif hp == 1 and qc < NQC - 1:
                        for et in range(8):
                            filler_q.append(functools.partial(emit_proj_a, qc, et))
                            filler_q.append(functools.partial(emit_proj_b, qc, et))
                    elif hp == 1:
                        for et in range(8):
                            filler_q.append(functools.partial(emit_proj3_b, et))
            flush(len(filler_q))

    nc.compile()
    return nc


def _get_nc():
    if "nc" not in _CACHE:
        _CACHE["nc"] = _build_program()
    return _CACHE["nc"]


def _shard_inputs(hidden_states, w_attn, w_proj):
    import ml_dtypes

    bf16 = ml_dtypes.bfloat16
    scale = 1.0 / np.sqrt(np.float32(HD))
    hidT_b = [
        np.ascontiguousarray(hidden_states[b].T).astype(bf16) for b in range(2)
    ]
    in_maps = []
    for c in range(N_CORES):
        b, g = divmod(c, 4)
        cs = slice(g * NH * HD, (g + 1) * NH * HD)
        wq = w_attn[:, 0:D][:, cs] * scale
        wk = w_attn[:, D : 2 * D][:, cs]
        wv = w_attn[:, 2 * D : 3 * D][:, cs]
        in_maps.append(
            {
                "hidT": hidT_b[b],
                "wqkv": np.ascontiguousarray(
                    np.concatenate([wq, wk, wv], axis=1)
                ).astype(bf16),
                "wp": np.ascontiguousarray(w_proj[cs, :]).astype(bf16),
            }
        )
    return in_maps


def run(hidden_states, w_attn, w_proj, trace=False):
    from concourse.bass_utils import run_bass_kernel_spmd

    nc = _get_nc()
    in_maps = _shard_inputs(hidden_states, w_attn, w_proj)
    res = run_bass_kernel_spmd(nc, in_maps, list(range(N_CORES)), trace=trace)
    parts = [res.results[c]["outT"].astype(np.float32).T for c in range(N_CORES)]
    out = np.stack(
        [
            parts[0] + parts[1] + parts[2] + parts[3],
            parts[4] + parts[5] + parts[6] + parts[7],
        ]
    ).astype(np.float32)
    return out, res


def kernel(hidden_states, w_attn, w_proj):
    out, _ = run(
        np.asarray(hidden_states), np.asarray(w_attn), np.asarray(w_proj)
    )
    return out
